# revision 26
# baseline (speedup 1.0000x reference)
"""ProdLSTM Trainium2 kernel: embedding lookup + one-hot features + LSTM(H=256) + linear head.

Self-contained: hardcodes all shapes. Shards batch 4096 -> 8 cores x 512,
replicates tables/weights, runs one SPMD Bass kernel, gathers output.
"""

import os
from contextlib import ExitStack

import numpy as np

import concourse.bass as bass
import concourse.tile as tile
from concourse import bacc
from concourse import mybir
from concourse import bass_utils
from concourse.masks import make_identity

F32 = mybir.dt.float32
F32R = mybir.dt.float32r
BF16 = mybir.dt.bfloat16
I32 = mybir.dt.int32

N_CORES = 8
B = 4096
BC = B // N_CORES  # 512 per core
L = 100
F = 26
EMB = 50
H = 256
G4 = 4 * H  # 1024
D_DYN = F + 7 + 24 + 28  # 85 dynamic features (dense + one-hot)
XD_K = 90  # xd rows: 0:59 one-hot, 59:64 zero pad, 64:90 dense
D_STAT = 21 + 3 * EMB  # 171 static features (aisle, user, prod, dept)
MAX_USERS, MAX_PRODUCTS, MAX_AISLES, MAX_DEPTS = 200000, 49688, 134, 21

P = 128
M_TILES = G4 // P  # 8
LCHUNK = 5  # timesteps per one-hot broadcast chunk


def r(ap):
    """View an fp32 AP as float32r for full-rate PE matmuls."""
    return ap.bitcast(F32R)


def build_program():
    nc = bacc.Bacc("TRN2", target_bir_lowering=False, debug=False)

    # ---- I/O ----
    d_inputs = nc.dram_tensor("inp", [BC, L * F], F32, kind="ExternalInput").ap()
    d_users = nc.dram_tensor("users", [BC, 1], I32, kind="ExternalInput").ap()
    d_prods = nc.dram_tensor("prods", [BC, 1], I32, kind="ExternalInput").ap()
    d_aisles = nc.dram_tensor("aisles", [BC, 1], I32, kind="ExternalInput").ap()
    d_depts = nc.dram_tensor("depts", [BC, 1], I32, kind="ExternalInput").ap()
    d_dows = nc.dram_tensor("dows", [BC, L], I32, kind="ExternalInput").ap()
    d_hours = nc.dram_tensor("hours", [BC, L], I32, kind="ExternalInput").ap()
    d_tzs = nc.dram_tensor("tzs", [BC, L], I32, kind="ExternalInput").ap()
    d_uemb = nc.dram_tensor("uemb", [MAX_USERS, EMB], F32, kind="ExternalInput").ap()
    d_pemb = nc.dram_tensor("pemb", [MAX_PRODUCTS, EMB], F32, kind="ExternalInput").ap()
    d_aemb = nc.dram_tensor("aemb", [MAX_AISLES, 21], F32, kind="ExternalInput").ap()
    d_demb = nc.dram_tensor("demb", [MAX_DEPTS, EMB], F32, kind="ExternalInput").ap()
    d_wih = nc.dram_tensor("wih", [G4, 256], F32, kind="ExternalInput").ap()
    d_whh = nc.dram_tensor("whh", [G4, H], F32, kind="ExternalInput").ap()
    d_bih = nc.dram_tensor("bih", [G4], F32, kind="ExternalInput").ap()
    d_bhh = nc.dram_tensor("bhh", [G4], F32, kind="ExternalInput").ap()
    d_wfin = nc.dram_tensor("wfin", [1, H], F32, kind="ExternalInput").ap()
    d_bfin = nc.dram_tensor("bfin", [1, 1], F32, kind="ExternalInput").ap()
    d_out = nc.dram_tensor("out", [L, BC], F32, kind="ExternalOutput").ap()

    # DRAM scratch: transposed dense inputs [l*26+f, b] and adjusted time idx
    # rows 0:100 dow, 100:200 hour+7, 200:300 tz+31 (fp32 values).
    d_inpT = nc.dram_tensor("inpT", [L * F, BC], BF16, kind="Internal").ap()
    d_timeT = nc.dram_tensor("timeT", [3 * L, BC], F32, kind="Internal").ap()

    with TileCtx(nc) as tc:
        build_kernel(nc, tc, locals())
    nc.compile()
    return nc


def TileCtx(nc):
    return tile.TileContext(nc)


def build_kernel(nc, tc, d):
    ctx = ExitStack()
    with ctx:
        _build_kernel(ctx, nc, tc, d)


def _build_kernel(ctx, nc, tc, d):
    Sig = mybir.ActivationFunctionType.Sigmoid
    Tanh = mybir.ActivationFunctionType.Tanh
    EQ = mybir.AluOpType.is_equal
    MUL = mybir.AluOpType.mult
    ADD = mybir.AluOpType.add

    # ---------- pools ----------
    const = ctx.enter_context(tc.tile_pool(name="const", bufs=1))

    # persistent tiles
    identity = const.tile([P, P], F32)
    make_identity(nc, identity[:])
    identity_r = const.tile([P, P], BF16)
    nc.vector.tensor_copy(identity_r[:], identity[:])
    w_dynT = const.tile([P, G4], BF16)  # rows 0:85 = W_ih^T[0:85]
    w_statTa = const.tile([P, G4], BF16)  # W_ih^T rows 85:213
    w_statTb = const.tile([P, G4], BF16)  # rows 0:43 = W_ih^T rows 213:256
    w_hhT0 = const.tile([P, G4], BF16)
    w_hhT1 = const.tile([P, G4], BF16)
    s2T = const.tile([P, M_TILES, BC], BF16)  # static proj + bias, per M tile
    bias_sb = const.tile([P, M_TILES], F32)
    iota_f = const.tile([P, 1], F32)
    wf_sb = const.tile([P, 2], BF16)  # W_final^T as two [128,1] K-tiles
    wf32_sb = const.tile([P, 2], F32)
    bf_sb = const.tile([1, 1], F32)

    zero_f32 = const.tile([P, G4], F32)
    nc.vector.memset(zero_f32[:], 0.0)

    # iota over partitions: value p at partition p (one-hot block base 0)
    iota_i = const.tile([P, 1], I32)
    nc.gpsimd.iota(iota_i[:], pattern=[[0, 1]], base=0, channel_multiplier=1)
    nc.vector.tensor_copy(iota_f[:], iota_i[:])

    # ---------- setup: weights ----------
    with tc.tile_pool(name="setup", bufs=2) as setup, \
         tc.tile_pool(name="setup_ps", bufs=4, space="PSUM") as setup_ps, \
         tc.tile_pool(name="stage", bufs=2) as stage:

        wih_sb = setup.tile([P, M_TILES, 256], F32, tag="wbig")
        nc.sync.dma_start(
            wih_sb[:], d["d_wih"].rearrange("(m p) d -> p m d", p=P))
        whh_sb = setup.tile([P, M_TILES, 256], F32, tag="wbig")
        nc.sync.dma_start(
            whh_sb[:], d["d_whh"].rearrange("(m p) d -> p m d", p=P))

        scrap_ps = setup_ps.tile([1, 1], F32, tag="scrap", bufs=1)

        def pe_sync(ap):
            # Tiny regular matmul whose only job is to advance PE's vector
            # clock past ap's producers (transpose-mode instructions can
            # carry at most one semaphore wait).
            nc.tensor.matmul(scrap_ps[0:1, 0:1], ap[:, 0:1], ap[:, 0:1],
                             start=True, stop=True, skip_group_check=True)

        def transpose_to(dst, m, src_ap, rows, dst_row0=0):
            ps = setup_ps.tile([P, P], F32, tag="tps")
            nc.tensor.transpose(ps[:rows, :], src_ap, identity[:])
            nc.vector.tensor_copy(
                dst[dst_row0:dst_row0 + rows, m * P:(m + 1) * P], ps[:rows, :])

        nc.vector.tensor_copy(w_dynT[:], zero_f32[:])
        pe_sync(identity)
        pe_sync(wih_sb[:, 0, :])
        pe_sync(whh_sb[:, 0, :])
        for m in range(M_TILES):
            transpose_to(w_dynT, m, wih_sb[:, m, 26:85], 59, dst_row0=0)
            transpose_to(w_dynT, m, wih_sb[:, m, 0:26], 26, dst_row0=64)
            transpose_to(w_statTa, m, wih_sb[:, m, 85:213], 128)
            transpose_to(w_statTb, m, wih_sb[:, m, 213:256], 43)
            transpose_to(w_hhT0, m, whh_sb[:, m, 0:128], 128)
            transpose_to(w_hhT1, m, whh_sb[:, m, 128:256], 128)

        # bias = b_ih + b_hh, laid out [128, m]
        btmp = setup.tile([P, M_TILES], F32, tag="btmp")
        btmp2 = setup.tile([P, M_TILES], F32, tag="btmp2")
        nc.sync.dma_start(btmp[:], d["d_bih"].rearrange("(m p) -> p m", p=P))
        nc.sync.dma_start(btmp2[:], d["d_bhh"].rearrange("(m p) -> p m", p=P))
        nc.vector.tensor_add(bias_sb[:], btmp[:], btmp2[:])

        nc.gpsimd.dma_start(wf_sb[:], d["d_wfin"].rearrange("o (k p) -> p (o k)", p=P))
        nc.sync.dma_start(wf32_sb[:], d["d_wfin"].rearrange("o (k p) -> p (o k)", p=P))
        nc.sync.dma_start(bf_sb[:], d["d_bfin"][:, :])

        # ---------- setup: embedding gather + static^T ----------
        staticTa = setup.tile([P, BC], BF16, tag="statTa")  # 128 feat rows
        staticTb = setup.tile([P, BC], BF16, tag="statTb")  # rows 0:43
        for t in range(BC // P):
            sl = slice(t * P, (t + 1) * P)
            gath = stage.tile([P, D_STAT], F32, tag="gath")
            for (dram_idx, dram_tab, c0, w) in (
                (d["d_aisles"], d["d_aemb"], 0, 21),
                (d["d_users"], d["d_uemb"], 21, EMB),
                (d["d_prods"], d["d_pemb"], 71, EMB),
                (d["d_depts"], d["d_demb"], 121, EMB),
            ):
                idxt = stage.tile([P, 1], I32, tag="idx")
                nc.sync.dma_start(idxt[:], dram_idx[sl, :])
                nc.gpsimd.indirect_dma_start(
                    out=gath[:, c0:c0 + w],
                    out_offset=None,
                    in_=dram_tab[:, :],
                    in_offset=bass.IndirectOffsetOnAxis(ap=idxt[:, :1], axis=0),
                )
            pe_sync(gath)
            ps = setup_ps.tile([P, P], F32, tag="tps")
            nc.tensor.transpose(ps[:, :], gath[:, 0:128], identity[:])
            nc.vector.tensor_copy(staticTa[:, sl], ps[:, :])
            ps = setup_ps.tile([P, P], F32, tag="tps")
            nc.tensor.transpose(ps[:43, :], gath[:, 128:171], identity[:])
            nc.vector.tensor_copy(staticTb[:43, sl], ps[:43, :])

        # s2T[m] = W_stat^T_m^T @ static^T + bias  (done as two K-tiles)
        for m in range(M_TILES):
            mc = slice(m * P, (m + 1) * P)
            ps = setup_ps.tile([P, BC], F32, tag="sps", bufs=2)
            nc.tensor.matmul(ps[:], w_statTa[:, mc], staticTa[:],
                             start=True, stop=False)
            nc.tensor.matmul(ps[:], w_statTb[0:43, mc], staticTb[0:43, :],
                             start=False, stop=True)
            nc.vector.tensor_scalar(
                out=s2T[:, m, :], in0=ps[:], scalar1=bias_sb[:, m:m + 1],
                scalar2=None, op0=ADD)

        # ---------- setup: transpose dense inputs to DRAM ----------
        inp_all = [setup.tile([P, L * F], F32, tag=f"inp{t}", name=f"inp{t}")
                   for t in range(4)]
        for t in range(4):
            nc.sync.dma_start(inp_all[t][:], d["d_inputs"][t * P:(t + 1) * P, :])
        for t in range(4):
            pe_sync(inp_all[t])
        CH = P
        nchunks = (L * F + CH - 1) // CH  # 21 (20x128 + 40)
        for c in range(nchunks):
            c0 = c * CH
            cw = min(CH, L * F - c0)
            row_sb = stage.tile([P, BC], BF16, tag="rowsb")
            for t in range(4):
                ps = setup_ps.tile([P, P], F32, tag="tps")
                nc.tensor.transpose(ps[:cw, :], inp_all[t][:, c0:c0 + cw],
                                    identity[:])
                nc.vector.tensor_copy(row_sb[:cw, t * P:(t + 1) * P], ps[:cw, :])
            nc.sync.dma_start(d["d_inpT"][c0:c0 + cw, :], row_sb[:cw, :])

        # ---------- setup: adjusted time indices, transposed, to DRAM ----------
        for fi, (dram_t, off) in enumerate(
                ((d["d_dows"], 0), (d["d_hours"], 7), (d["d_tzs"], 31))):
            trow = stage.tile([P, BC], F32, tag="trow")
            for t in range(4):
                ti = stage.tile([P, L], I32, tag="ti")
                nc.sync.dma_start(ti[:], dram_t[t * P:(t + 1) * P, :])
                tf = stage.tile([P, L], F32, tag="tf")
                if off:
                    nc.vector.tensor_scalar(
                        out=tf[:], in0=ti[:], scalar1=float(off), scalar2=None,
                        op0=ADD)
                else:
                    nc.vector.tensor_copy(tf[:], ti[:])
                ps = setup_ps.tile([P, P], F32, tag="tps")
                nc.tensor.transpose(ps[:L, :], tf[:, :], identity[:])
                nc.vector.tensor_copy(trow[:L, t * P:(t + 1) * P], ps[:L, :])
            nc.sync.dma_start(d["d_timeT"][fi * L:(fi + 1) * L, :], trow[:L, :])

    # ---------- main loop pools ----------
    gps = ctx.enter_context(tc.tile_pool(name="gpsum", bufs=8, space="PSUM"))
    xdp = ctx.enter_context(tc.tile_pool(name="xdyn", bufs=4))
    bcp = ctx.enter_context(tc.tile_pool(name="bcast", bufs=3))
    gatep = ctx.enter_context(tc.tile_pool(name="gates", bufs=3))
    hcp = ctx.enter_context(tc.tile_pool(name="hc", bufs=3))
    tmpp = ctx.enter_context(tc.tile_pool(name="tmps", bufs=3))

    h_prev = hcp.tile([P, 2 * BC], BF16, tag="h")
    c_prev = hcp.tile([P, 2 * BC], F32, tag="c")
    nc.vector.tensor_copy(h_prev[:], zero_f32[:, 0:2 * BC])
    nc.vector.memset(c_prev[:], 0.0)

    def emit_y(h_tile, l_out):
        # y_{l_out} = w_final . h_{l_out} + b_final, streamed to DRAM
        psy = gps.tile([P, BC], F32, tag="g", name=f"psy{l_out}")
        nc.tensor.matmul(psy[0:1, :], wf_sb[:, 0:1], h_tile[:, 0:BC],
                         start=True, stop=False)
        nc.tensor.matmul(psy[0:1, :], wf_sb[:, 1:2], h_tile[:, BC:],
                         start=False, stop=True)
        ystage = tmpp.tile([1, BC], F32, tag="yst", bufs=4,
                           name=f"yst{l_out}")
        nc.vector.tensor_scalar(
            out=ystage[0:1, :], in0=psy[0:1, :], scalar1=bf_sb[0:1, 0:1],
            scalar2=None, op0=ADD)
        nc.sync.dma_start(d["d_out"][l_out:l_out + 1, :], ystage[0:1, :])

    # bank order: i, g, f, o — the cell update needs i and g first
    BANK_MS = (0, 1, 4, 5, 2, 3, 6, 7)

    bc_holder = [None]

    def build_xd(l):
        lb = l % LCHUNK
        if lb == 0:
            nch = min(LCHUNK, L - l)
            bc_tile = bcp.tile([P, LCHUNK, BC], F32, tag="bc",
                               name=f"bc{l}")
            # broadcast adjusted time index rows across partition blocks
            for (p0, pw, base) in ((0, 7, 0), (7, 24, L), (31, 28, 2 * L)):
                s_ap = d["d_timeT"][base + l: base + l + nch, :]
                src_b = bass.AP(
                    tensor=s_ap.tensor, offset=s_ap.offset,
                    ap=[[0, pw]] + s_ap.ap)
                nc.gpsimd.dma_start(out=bc_tile[p0:p0 + pw, 0:nch, :],
                                    in_=src_b)
            bc_holder[0] = bc_tile
        xd = xdp.tile([P, BC], BF16, tag="xd", name=f"xd{l}")
        nc.sync.dma_start(xd[64:64 + F, :], d["d_inpT"][l * F:(l + 1) * F, :])
        # one-hot rows 0:59: compare broadcast idx against partition iota
        nc.vector.tensor_scalar(
            out=xd[0:64, :], in0=bc_holder[0][0:64, lb, :],
            scalar1=iota_f[0:64, 0:1], scalar2=None, op0=EQ)
        return xd

    xd_cur = build_xd(0)
    for l in range(L):
        # gates: 8 PSUM banks, 4 matmuls each. All h-independent matmuls
        # for this step are issued first so the PE never stalls on the
        # previous step's nonlinearity chain.
        g_ps = {}
        for m in BANK_MS:
            mc = slice(m * P, (m + 1) * P)
            ps = gps.tile([P, BC], F32, tag="g", name=f"g{l}_{m}")
            nc.tensor.matmul(ps[:], identity_r[:], s2T[:, m, :],
                             start=True, stop=False, skip_group_check=True)
            nc.tensor.matmul(ps[:], w_dynT[0:XD_K, mc], xd_cur[0:XD_K, :],
                             start=False, stop=False, skip_group_check=True)
            g_ps[m] = ps
        # next step's dynamic features: issued ahead of this step's cell
        # chain so the DVE produces them before it gets busy.
        if l + 1 < L:
            xd_next = build_xd(l + 1)
        for m in BANK_MS:
            mc = slice(m * P, (m + 1) * P)
            nc.tensor.matmul(g_ps[m][:], w_hhT0[:, mc], h_prev[:, 0:BC],
                             start=False, stop=False, skip_group_check=True)
        for m in BANK_MS:
            mc = slice(m * P, (m + 1) * P)
            nc.tensor.matmul(g_ps[m][:], w_hhT1[:, mc],
                             h_prev[:, BC:2 * BC],
                             start=False, stop=True, skip_group_check=True)
        if l > 0:
            emit_y(h_prev, l - 1)

        it = gatep.tile([P, 2 * BC], BF16, tag="ig")
        gt = gatep.tile([P, 2 * BC], BF16, tag="gg")
        ft = gatep.tile([P, 2 * BC], BF16, tag="fg")
        ot = gatep.tile([P, 2 * BC], BF16, tag="og")
        for gtile, fn, (m0, m1) in ((it, Sig, (0, 1)), (gt, Tanh, (4, 5)),
                                    (ft, Sig, (2, 3))):
            nc.scalar.activation(out=gtile[:, 0:BC], in_=g_ps[m0][:], func=fn)
            nc.scalar.activation(out=gtile[:, BC:], in_=g_ps[m1][:], func=fn)
        nc.scalar.activation(out=ot[:, 0:BC], in_=g_ps[6][:], func=Sig)
        nc.scalar.activation(out=ot[:, BC:], in_=g_ps[7][:], func=Sig)

        # cell update split by H-halves: half 0 finishes early and feeds
        # the next step's first-K recurrent matmuls; tanh halves overlap
        # the other half's DVE work.
        c_new = hcp.tile([P, 2 * BC], F32, tag="c")
        tmp = tmpp.tile([P, 2 * BC], BF16, tag="tmp")
        tct = tmpp.tile([P, 2 * BC], BF16, tag="tct")
        h_new = hcp.tile([P, 2 * BC], BF16, tag="h")
        h0, h1 = slice(0, BC), slice(BC, 2 * BC)
        nc.vector.tensor_tensor(out=tmp[:, h0], in0=it[:, h0], in1=gt[:, h0],
                                op=MUL)
        nc.vector.tensor_tensor(out=c_new[:, h0], in0=ft[:, h0],
                                in1=c_prev[:, h0], op=MUL)
        nc.vector.tensor_tensor(out=c_new[:, h0], in0=c_new[:, h0],
                                in1=tmp[:, h0], op=ADD)
        nc.scalar.activation(out=tct[:, h0], in_=c_new[:, h0], func=Tanh)
        nc.vector.tensor_tensor(out=tmp[:, h1], in0=it[:, h1], in1=gt[:, h1],
                                op=MUL)
        nc.vector.tensor_tensor(out=c_new[:, h1], in0=ft[:, h1],
                                in1=c_prev[:, h1], op=MUL)
        nc.vector.tensor_tensor(out=h_new[:, h0], in0=ot[:, h0],
                                in1=tct[:, h0], op=MUL)
        nc.vector.tensor_tensor(out=c_new[:, h1], in0=c_new[:, h1],
                                in1=tmp[:, h1], op=ADD)
        nc.scalar.activation(out=tct[:, h1], in_=c_new[:, h1], func=Tanh)
        nc.vector.tensor_tensor(out=h_new[:, h1], in0=ot[:, h1],
                                in1=tct[:, h1], op=MUL)

        h_prev, c_prev = h_new, c_new
        if l + 1 < L:
            xd_cur = xd_next

    emit_y(h_prev, L - 1)



def _ensure_axon_profile_hook():
    """Register the NTFF profile hook (test/profiling path only).

    The agent image's antenv package lacks axon_hooks; replicate it so
    run_bass_kernel_spmd(trace=True) can capture NTFF profiles.
    """
    import sys
    import types
    try:
        from antenv.axon_hooks import get_axon_ntff_profile_hook  # noqa: F401
        return
    except ImportError:
        pass
    mod = types.ModuleType("antenv.axon_hooks")
    holder = {"hook": None}
    mod.set_axon_ntff_profile_hook = lambda h: holder.__setitem__("hook", h)
    mod.get_axon_ntff_profile_hook = lambda: holder["hook"]
    sys.modules["antenv.axon_hooks"] = mod
    import antenv
    antenv.axon_hooks = mod
    from trn_agent_boot.trn_boot import _ntff_profile_via_ctypes
    mod.set_axon_ntff_profile_hook(
        _ntff_profile_via_ctypes("/opt/axon/libaxon_pjrt.so"))


_CACHE = {}


def _get_program():
    if "nc" not in _CACHE:
        _CACHE["nc"] = build_program()
    return _CACHE["nc"]


def run(inputs, trace=False):
    if trace:
        try:
            _ensure_axon_profile_hook()
        except Exception as e:
            print(f"profile hook unavailable: {e}")
    nc = _get_program()
    f32 = lambda a: np.ascontiguousarray(np.asarray(a), dtype=np.float32)
    i32 = lambda a: np.ascontiguousarray(np.asarray(a), dtype=np.int32)

    inp = f32(inputs["inputs"]).reshape(N_CORES, BC, L * F)
    users = i32(inputs["users"]).reshape(N_CORES, BC, 1)
    prods = i32(inputs["products"]).reshape(N_CORES, BC, 1)
    aisles = i32(inputs["aisles"]).reshape(N_CORES, BC, 1)
    depts = i32(inputs["depts"]).reshape(N_CORES, BC, 1)
    dows = i32(inputs["dows"]).reshape(N_CORES, BC, L)
    hours = i32(inputs["hours"]).reshape(N_CORES, BC, L)
    tzs = i32(inputs["tzs"]).reshape(N_CORES, BC, L)
    uemb = f32(inputs["user_emb"])
    pemb = f32(inputs["prod_emb"])
    aemb = f32(inputs["aisle_emb"])
    demb = f32(inputs["dept_emb"])
    wih = f32(inputs["W_ih"])
    whh = f32(inputs["W_hh"])
    bih = f32(inputs["b_ih"])
    bhh = f32(inputs["b_hh"])
    wfin = f32(inputs["W_final"])
    bfin = f32(inputs["b_final"]).reshape(1, 1)

    in_maps = []
    for c in range(N_CORES):
        in_maps.append({
            "inp": inp[c], "users": users[c], "prods": prods[c],
            "aisles": aisles[c], "depts": depts[c], "dows": dows[c],
            "hours": hours[c], "tzs": tzs[c],
            "uemb": uemb, "pemb": pemb, "aemb": aemb, "demb": demb,
            "wih": wih, "whh": whh, "bih": bih, "bhh": bhh,
            "wfin": wfin, "bfin": bfin,
        })

    res = bass_utils.run_bass_kernel_spmd(
        nc, in_maps, core_ids=list(range(N_CORES)), trace=trace)
    outs = [res.results[c]["out"] for c in range(N_CORES)]  # each [L, BC]
    full = np.concatenate([o.T for o in outs], axis=0)  # [B, L]
    return full, res.exec_time_ns


def kernel(**inputs):
    out, _ = run(inputs, trace=os.environ.get("BASS_KERNEL_TRACE", "") == "1")
    return out
